# revision 25
# baseline (speedup 1.0000x reference)
"""Multi-head attention (B=4, S=2048, D=1024, H=16, dk=dv=64) on 8 TRN2 cores.

Sharding: core c = 2*b + hg handles batch b = c//2 and heads
[hg*8, hg*8+8). Each core computes a partial output
(its 8 heads' contribution through Wo); the host adds the two partials
per batch.

Per-core device pipeline (matmul inputs bf16, PSUM accumulation fp32,
softmax sums/reciprocal fp32):
  1. q(qb=0) projection first (shortest path to attention), then khT
     projections (pair layout: h0 dk on partitions 0-63, h1 on 64-127),
     then vh projection per key-chunk as [128, 8*128] bf16 with a
     mask/ones column appended per head (masked keys zeroed; cols 65-127
     zero). q(qb+1) projections are interleaved into attention qb.
  2. scores^T per head pair via 64x128 PE row tiling: per key-chunk one
     [128, 1024] PSUM tile holds h0 scores (cols 0-511, tile (0,0)) and
     h1 scores (cols 512-1023, tile (64,0)); the two matmuls co-stream
     in the PE array (separate PSUM banks).
  3. exp on ScalarE PSUM->SBUF bf16, one [128, 1024] ACTIVATE per chunk.
  4. mix^T + softmax sums in one matmul: lhsT = vh block [128 keys,
     128] (col 64 = mask/ones), rhs = exp half [128, 512]; PSUM
     accumulation over the 16 chunks (mixP for h0, mixR for h1).
  5. normalize: Z row (partition 64) -> bf16 SBUF, K=1 PE matmul
     broadcasts it to partitions 0-63, reciprocal_approx_fast at base
     partition 0 (custom-DVE ops misbehave at base partition 64),
     multiply mix rows by 1/Z (bf16 out). h1's normalized tile is
     DMA-shifted to partitions 64-127 so each pair's mix^T is one
     [128, 512] tile (e on partitions).
  6. out += mixT_norm.T @ Wo: dense K=128 bf16 matmuls accumulating over
     the 4 pairs; DVE evac fp32 -> DMA to HBM.
"""

import numpy as np

B, S, D = 4, 2048, 1024
H, DK, DV = 16, 64, 64
HC = 8          # heads per core
NP = HC // 2    # head pairs per core
NCORES = 8
NC_CHUNKS = D // 128    # 8 contraction chunks over D
NKC = S // 128          # 16 key chunks
NQB = S // 512          # 4 query blocks
VW = HC * 128           # vh storage: 128 cols per head (dv | mask | zeros)

_COMPILED = {}


def _build_nc():
    import concourse.tile as tile
    from concourse import bacc, mybir
    from contextlib import ExitStack

    F32 = mybir.dt.float32
    BF16 = mybir.dt.bfloat16
    EXP = mybir.ActivationFunctionType.Exp

    nc = bacc.Bacc("TRN2", target_bir_lowering=False, debug=False,
                   num_devices=NCORES)

    qT = nc.dram_tensor("qT", [D, S], BF16, kind="ExternalInput").ap()
    kT = nc.dram_tensor("kT", [D, S], BF16, kind="ExternalInput").ap()
    vT = nc.dram_tensor("vT", [D, S], BF16, kind="ExternalInput").ap()
    wq = nc.dram_tensor("wq", [D, HC * DK], BF16, kind="ExternalInput").ap()
    wk = nc.dram_tensor("wk", [D, HC * DK], BF16, kind="ExternalInput").ap()
    wv = nc.dram_tensor("wv", [D, HC * DV], BF16, kind="ExternalInput").ap()
    wo = nc.dram_tensor("wo", [HC * DV, D], BF16, kind="ExternalInput").ap()
    maskr = nc.dram_tensor("maskr", [128, NKC], F32, kind="ExternalInput").ap()
    out = nc.dram_tensor("out", [S, D], F32, kind="ExternalOutput").ap()

    with tile.TileContext(nc) as tc:
        with ExitStack() as ctx:
            const_pool = ctx.enter_context(tc.tile_pool(name="const", bufs=1))
            w_pool = ctx.enter_context(tc.tile_pool(name="weights", bufs=1))
            act_pool = ctx.enter_context(tc.tile_pool(name="acts", bufs=1))
            st_pool = ctx.enter_context(
                tc.tile_pool(name="stage", bufs=32))
            vt_pool = ctx.enter_context(tc.tile_pool(name="vtpool", bufs=1))
            # PSUM: pj(2, shared with bc) + sc(2x2) + mx(2) = 8 banks
            pj_pool = ctx.enter_context(
                tc.tile_pool(name="pjpsum", bufs=2, space="PSUM"))
            sc_pool = ctx.enter_context(
                tc.tile_pool(name="scpsum", bufs=2, space="PSUM"))
            mx_pool = ctx.enter_context(
                tc.tile_pool(name="mxpsum", bufs=1, space="PSUM"))
            exp_pool = ctx.enter_context(tc.tile_pool(name="exp", bufs=4))
            norm_pool = ctx.enter_context(tc.tile_pool(name="norm",
                                                       bufs=2 * NP))
            tmp_pool = ctx.enter_context(tc.tile_pool(name="tmp", bufs=2))
            out_pool = ctx.enter_context(tc.tile_pool(name="outsb", bufs=4))

            # weight tiles (DMAs issued in need-order below)
            wq_sb = w_pool.tile([128, NC_CHUNKS * 512], BF16, tag="wq")
            wk_sb = w_pool.tile([128, NC_CHUNKS * 512], BF16, tag="wk")
            wv_sb = w_pool.tile([128, NC_CHUNKS * 512], BF16, tag="wv")
            wo_sb = w_pool.tile([128, NP * 1024], BF16, tag="wo")

            mask_sb = const_pool.tile([128, NKC], F32)
            ones_sb = const_pool.tile([128, 64], BF16)
            e65_sb = const_pool.tile([128, DV + 1], BF16)

            # persistent activations
            qhT = [act_pool.tile([128, S], BF16, tag=f"qhT{p}", name=f"qhT{p}")
                   for p in range(NP)]
            khT = [act_pool.tile([128, S], BF16, tag=f"khT{p}",
                                 name=f"khT{p}") for p in range(NP)]
            vhs = [act_pool.tile([128, VW], BF16, tag=f"vh{t}", name=f"vh{t}")
                   for t in range(NKC)]

            # ---- issue order: q(qb0) path first ----
            for c in range(NC_CHUNKS):
                nc.sync.dma_start(wq_sb[:, c * 512:(c + 1) * 512],
                                  wq[c * 128:(c + 1) * 128, :])
            nc.sync.dma_start(mask_sb[:], maskr[:])
            nc.vector.memset(ones_sb[:], 1.0)
            nc.vector.memset(e65_sb[64:65, :], 1.0)
            for t in range(NKC):
                nc.vector.memset(vhs[t][:, :], 0.0)

            def stage_block(src, qb):
                stg = []
                for c in range(NC_CHUNKS):
                    t = st_pool.tile([128, 512], BF16, tag="stage",
                                     name=f"stg{c}")
                    nc.sync.dma_start(
                        t[:], src[c * 128:(c + 1) * 128,
                                  qb * 512:(qb + 1) * 512])
                    stg.append(t)
                return stg

            def proj_pair(stg, wsb, p, dst, qb):
                ps = pj_pool.tile([128, 512], F32, tag="pj", name="pps")
                for c in range(NC_CHUNKS):
                    nc.tensor.matmul(
                        ps[:],
                        lhsT=wsb[:, c * 512 + p * 128:c * 512 + (p + 1) * 128],
                        rhs=stg[c][:],
                        start=(c == 0), stop=(c == NC_CHUNKS - 1))
                nc.vector.tensor_copy(
                    dst[p][:, qb * 512:(qb + 1) * 512], ps[:])

            def vproj_granule(t):
                # v projection chunk t (with mask fold + ones col)
                ps = pj_pool.tile([128, 512], F32, tag="pj", name="vps")
                for c in range(NC_CHUNKS):
                    nc.tensor.matmul(
                        ps[:],
                        lhsT=vt_sb[c][:, t * 128:(t + 1) * 128],
                        rhs=wv_sb[:, c * 512:(c + 1) * 512],
                        start=(c == 0), stop=(c == NC_CHUNKS - 1))
                # masked copy into vh store (strided per head) + mask col
                dst_dv = vhs[t][:, 0:VW].rearrange(
                    "p (h x) -> p h x", x=128)[:, :, 0:DV]
                src_dv = ps[:].rearrange("p (h x) -> p h x", x=DV)
                nc.vector.tensor_scalar_mul(dst_dv, src_dv,
                                            mask_sb[:, t:t + 1])
                dst_m = vhs[t][:, 0:VW].rearrange(
                    "p (h x) -> p h x", x=128)[:, :, DV:DV + 1]
                src_m = ones_sb[:, 0:HC].rearrange("p (h x) -> p h x", x=1)
                nc.vector.tensor_scalar_mul(dst_m, src_m,
                                            mask_sb[:, t:t + 1])

            # q(qb=0) projection
            qstg = stage_block(qT, 0)
            for p in range(NP):
                proj_pair(qstg, wq_sb, p, qhT, 0)

            # k block 0 for all pairs upfront; kb1-3 JIT per pair below
            for c in range(NC_CHUNKS):
                nc.sync.dma_start(wk_sb[:, c * 512:(c + 1) * 512],
                                  wk[c * 128:(c + 1) * 128, :])
            kstg = {0: stage_block(kT, 0)}
            for p in range(NP):
                proj_pair(kstg[0], wk_sb, p, khT, 0)

            # v/o weights + vT staged token-major (early tokens first),
            # k stages for kb1-3 interleaved behind them
            for c in range(NC_CHUNKS):
                nc.sync.dma_start(wv_sb[:, c * 512:(c + 1) * 512],
                                  wv[c * 128:(c + 1) * 128, :])
            vt_sb = [vt_pool.tile([128, S], BF16, tag=f"vt{c}", name=f"vt{c}")
                     for c in range(NC_CHUNKS)]
            for tb in range(NQB):
                for c in range(NC_CHUNKS):
                    nc.sync.dma_start(
                        vt_sb[c][:, tb * 512:(tb + 1) * 512],
                        vT[c * 128:(c + 1) * 128, tb * 512:(tb + 1) * 512])
                if tb + 1 < NQB:
                    kstg[tb + 1] = stage_block(kT, tb + 1)
            for p in range(NP):
                nc.sync.dma_start(wo_sb[:, p * 1024:(p + 1) * 1024],
                                  wo[p * 128:(p + 1) * 128, :])

            # ---- attention + output projection ----
            # Software-pipelined over flat units u = (qb, p, kc): the
            # scores+exp issue runs LOOK units ahead of the mix issue so
            # ScalarE keeps exp-ing across pair boundaries while the PE
            # absorbs normalize/proj/Wo work in its slack.
            LOOK = 2
            units = [(qb, p, kc) for qb in range(NQB) for p in range(NP)
                     for kc in range(NKC)]
            pend = {}
            mix_tiles = {}
            stg_nxt = {}
            normT = {qb: [] for qb in range(NQB)}

            def issue_sc(u):
                qb, p, kc = u
                qsl = slice(qb * 512, (qb + 1) * 512)
                ksl = slice(kc * 128, (kc + 1) * 128)
                scP = sc_pool.tile([128, 1024], F32, tag="sc")
                # 64x128 PE row tiling: both heads co-stream
                nc.tensor.matmul(
                    scP[:, 0:512],
                    lhsT=khT[p][0:64, ksl], rhs=qhT[p][0:64, qsl],
                    start=True, stop=True, tile_position=(0, 0))
                nc.tensor.matmul(
                    scP[:, 512:1024],
                    lhsT=khT[p][64:128, ksl], rhs=qhT[p][64:128, qsl],
                    start=True, stop=True, tile_position=(64, 0))
                exP = exp_pool.tile([128, 1024], BF16, tag="exp")
                nc.scalar.activation(exP[:], scP[:], EXP)
                pend[u] = exP

            def issue_mix(u):
                qb, p, kc = u
                h0, h1 = 2 * p, 2 * p + 1
                if kc == 0:
                    mix_tiles[(qb, p)] = mx_pool.tile(
                        [128, 1024], F32, tag="mx", name="mixPR")
                mixPR = mix_tiles[(qb, p)]
                exP = pend.pop(u)
                va = vhs[kc]
                st = (kc == 0)
                sp = (kc == NKC - 1)
                nc.tensor.matmul(
                    mixPR[:, 0:512], lhsT=va[:, h0 * 128:(h0 + 1) * 128],
                    rhs=exP[:, 0:512], start=st, stop=sp)
                nc.tensor.matmul(
                    mixPR[:, 512:1024], lhsT=va[:, h1 * 128:(h1 + 1) * 128],
                    rhs=exP[:, 512:1024], start=st, stop=sp)

            def normalize(qb, p):
                # evac mix PSUM to SBUF immediately (frees the banks fast)
                mixPR = mix_tiles.pop((qb, p))
                mloc = tmp_pool.tile([128, 1024], F32, tag="mloc")
                nc.vector.tensor_copy(mloc[:], mixPR[:])
                nt = norm_pool.tile([128, 512], BF16, tag="norm")
                normT[qb].append(nt)
                zrow = tmp_pool.tile([128, 1024], BF16, tag="zrow")
                nc.vector.tensor_copy(zrow[64:65, :], mloc[64:65, :])

                def finish():
                    # Z row bf16 -> K=1 PE bcast, reciprocal at base
                    # partition 0 (custom-DVE ops misbehave at base
                    # partition 64), scale mix rows
                    bc0 = pj_pool.tile([128, 512], F32, tag="pj")
                    bc1 = pj_pool.tile([128, 512], F32, tag="pj")
                    nc.tensor.matmul(
                        bc0[0:64, :], lhsT=e65_sb[64:65, 0:64],
                        rhs=zrow[64:65, 0:512], start=True, stop=True,
                        tile_position=(64, 0))
                    nc.tensor.matmul(
                        bc1[0:64, :], lhsT=e65_sb[64:65, 0:64],
                        rhs=zrow[64:65, 512:1024], start=True, stop=True,
                        tile_position=(64, 0))
                    rec0 = tmp_pool.tile([64, 512], F32, tag="rec")
                    rec1 = tmp_pool.tile([64, 512], F32, tag="rec")
                    nc.vector.reciprocal_approx_fast(rec0[:], bc0[0:64, :])
                    nc.vector.reciprocal_approx_fast(rec1[:], bc1[0:64, :])
                    nc.vector.tensor_mul(nt[0:64, :], mloc[0:64, 0:512],
                                         rec0[:])
                    sh1 = tmp_pool.tile([64, 512], BF16, tag="sh1")
                    nc.vector.tensor_mul(sh1[:], mloc[0:64, 512:1024],
                                         rec1[:])
                    nc.sync.dma_start(nt[64:128, :], sh1[:])
                return finish

            deferred = []

            def wo_piece(qb, tt, dh):
                def run():
                    wps = pj_pool.tile([128, 512], F32, tag="pj",
                                       name="wps")
                    for p in range(NP):
                        nc.tensor.matmul(
                            wps[:],
                            lhsT=normT[qb][p][:, tt * 128:(tt + 1) * 128],
                            rhs=wo_sb[:, p * 1024 + dh * 512:
                                      p * 1024 + (dh + 1) * 512],
                            start=(p == 0), stop=(p == NP - 1))
                    osb = out_pool.tile([128, 512], F32, tag="osb",
                                        name="osb")
                    nc.vector.tensor_copy(osb[:], wps[:])
                    nc.sync.dma_start(
                        out[qb * 512 + tt * 128:qb * 512 + (tt + 1) * 128,
                            dh * 512:(dh + 1) * 512], osb[:])
                return run

            def proj_piece(qb, p, cs):
                def run():
                    stg = stg_nxt[qb]
                    key = ("pps", qb, p)
                    if cs == 0:
                        mix_tiles[key] = pj_pool.tile(
                            [128, 512], F32, tag="pj", name="pps")
                    ps = mix_tiles[key]
                    for c in (cs, cs + 1):
                        nc.tensor.matmul(
                            ps[:],
                            lhsT=wq_sb[:, c * 512 + p * 128:
                                       c * 512 + (p + 1) * 128],
                            rhs=stg[c][:],
                            start=(c == 0), stop=(c == NC_CHUNKS - 1))
                    if cs + 2 == NC_CHUNKS:
                        nc.vector.tensor_copy(
                            qhT[p][:, qb * 512:(qb + 1) * 512], ps[:])
                        del mix_tiles[key]
                return run

            kdone = {(p, 0) for p in range(NP)}
            for i, u in enumerate(units):
                if i == 0:
                    for j in range(LOOK):
                        issue_sc(units[j])
                if u[0] == 0 and u[1] == 0:
                    vproj_granule(u[2])
                issue_mix(u)
                if i + LOOK < len(units):
                    nxt = units[i + LOOK]
                    if nxt[0] == 0 and nxt[2] % 4 == 0:
                        kk = (nxt[1], nxt[2] // 4)
                        if kk not in kdone:
                            kdone.add(kk)
                            proj_pair(kstg[kk[1]], wk_sb, kk[0], khT, kk[1])
                    issue_sc(nxt)
                if deferred:
                    deferred.pop(0)()
                qb, p, kc = u
                if p == 0 and kc == 14 and qb + 1 < NQB:
                    stg_nxt[qb + 1] = stage_block(qT, qb + 1)
                if kc == NKC - 1:
                    deferred.append(normalize(qb, p))
                    if qb + 1 < NQB:
                        deferred.extend(
                            proj_piece(qb + 1, p, cs)
                            for cs in range(0, NC_CHUNKS, 2))
                    if p == NP - 1:
                        deferred.extend(wo_piece(qb, tt, dh)
                                        for tt in range(4) for dh in range(2))
            while deferred:
                deferred.pop(0)()

    nc.compile()
    return nc


def _get_nc():
    if "nc" not in _COMPILED:
        _COMPILED["nc"] = _build_nc()
    return _COMPILED["nc"]


def _shard_inputs(q, k, v, mask, Wq, Wk, Wv, Wo):
    """Build the per-core input maps (host-side layout prep)."""
    import ml_dtypes

    bf16 = ml_dtypes.bfloat16
    in_maps = []
    maskf = np.asarray(mask).astype(np.float32)
    q = np.asarray(q, np.float32)
    k = np.asarray(k, np.float32)
    v = np.asarray(v, np.float32)
    Wq = np.asarray(Wq, np.float32)
    Wk = np.asarray(Wk, np.float32)
    Wv = np.asarray(Wv, np.float32)
    Wo = np.asarray(Wo, np.float32)
    scale = np.float32(1.0 / np.sqrt(DK))
    for c in range(NCORES):
        b, hg = c // 2, c % 2
        hs = hg * HC
        m = {
            "qT": np.ascontiguousarray(q[b].T).astype(bf16),
            "kT": np.ascontiguousarray(k[b].T).astype(bf16),
            "vT": np.ascontiguousarray(v[b].T).astype(bf16),
            # head-major col blocks; fold 1/sqrt(dk) into Wq
            "wq": np.ascontiguousarray(
                Wq[hs:hs + HC].transpose(1, 0, 2).reshape(D, HC * DK) * scale
            ).astype(bf16),
            "wk": np.ascontiguousarray(
                Wk[hs:hs + HC].transpose(1, 0, 2).reshape(D, HC * DK)
            ).astype(bf16),
            "wv": np.ascontiguousarray(
                Wv[hs:hs + HC].transpose(1, 0, 2).reshape(D, HC * DV)
            ).astype(bf16),
            "wo": np.ascontiguousarray(Wo[hs * DV:(hs + HC) * DV]).astype(bf16),
            "maskr": np.ascontiguousarray(
                maskf[b].reshape(NKC, 128).T).astype(np.float32),
        }
        in_maps.append(m)
    return in_maps


def kernel(q, k, v, mask, Wq, Wk, Wv, Wo, _trace=False):
    from concourse.bass_utils import run_bass_kernel_spmd

    nc = _get_nc()
    in_maps = _shard_inputs(q, k, v, mask, Wq, Wk, Wv, Wo)
    res = run_bass_kernel_spmd(nc, in_maps, list(range(NCORES)),
                               trace=_trace)
    out = np.zeros((B, S, D), np.float32)
    for c in range(NCORES):
        out[c // 2] += res.results[c]["out"]
    if _trace:
        _COMPILED["last_result"] = res
    return out


# revision 30
# speedup vs baseline: 1.0341x; 1.0341x over previous
"""Multi-head attention (B=4, S=2048, D=1024, H=16, dk=dv=64) on 8 TRN2 cores.

Sharding: core c = 2*b + hg handles batch b = c//2 and heads
[hg*8, hg*8+8). Each core computes a partial output
(its 8 heads' contribution through Wo); the host adds the two partials
per batch.

Per-core device pipeline (matmul inputs bf16, PSUM accumulation fp32,
softmax sums/reciprocal fp32):
  1. q(qb=0) projection first (shortest path to attention), then khT
     projections (pair layout: h0 dk on partitions 0-63, h1 on 64-127),
     then vh projection per key-chunk as [128, 8*128] bf16 with a
     mask/ones column appended per head (masked keys zeroed; cols 65-127
     zero). q(qb+1) projections are interleaved into attention qb.
  2. scores^T per head pair via 64x128 PE row tiling: per key-chunk one
     [128, 1024] PSUM tile holds h0 scores (cols 0-511, tile (0,0)) and
     h1 scores (cols 512-1023, tile (64,0)); the two matmuls co-stream
     in the PE array (separate PSUM banks).
  3. exp on ScalarE PSUM->SBUF bf16, one [128, 1024] ACTIVATE per chunk.
  4. mix^T + softmax sums in one matmul: lhsT = vh block [128 keys,
     128] (col 64 = mask/ones), rhs = exp half [128, 512]; PSUM
     accumulation over the 16 chunks (mixP for h0, mixR for h1).
  5. normalize: Z row (partition 64) -> bf16 SBUF, K=1 PE matmul
     broadcasts it to partitions 0-63, reciprocal_approx_fast at base
     partition 0 (custom-DVE ops misbehave at base partition 64),
     multiply mix rows by 1/Z (bf16 out). h1's normalized tile is
     DMA-shifted to partitions 64-127 so each pair's mix^T is one
     [128, 512] tile (e on partitions).
  6. out += mixT_norm.T @ Wo: dense K=128 bf16 matmuls accumulating over
     the 4 pairs; DVE evac fp32 -> DMA to HBM.
"""

import numpy as np

B, S, D = 4, 2048, 1024
H, DK, DV = 16, 64, 64
HC = 8          # heads per core
NP = HC // 2    # head pairs per core
NCORES = 8
NC_CHUNKS = D // 128    # 8 contraction chunks over D
NKC = S // 128          # 16 key chunks
NQB = S // 512          # 4 query blocks
VW = HC * 128           # vh storage: 128 cols per head (dv | mask | zeros)

_COMPILED = {}


def _build_nc():
    import concourse.tile as tile
    from concourse import bacc, mybir
    from contextlib import ExitStack

    F32 = mybir.dt.float32
    BF16 = mybir.dt.bfloat16
    EXP = mybir.ActivationFunctionType.Exp

    nc = bacc.Bacc("TRN2", target_bir_lowering=False, debug=False,
                   num_devices=NCORES)

    qT = nc.dram_tensor("qT", [D, S], BF16, kind="ExternalInput").ap()
    kT = nc.dram_tensor("kT", [D, S], BF16, kind="ExternalInput").ap()
    vT = nc.dram_tensor("vT", [D, S], BF16, kind="ExternalInput").ap()
    wq = nc.dram_tensor("wq", [D, HC * DK], BF16, kind="ExternalInput").ap()
    wk = nc.dram_tensor("wk", [D, HC * DK], BF16, kind="ExternalInput").ap()
    wv = nc.dram_tensor("wv", [D, HC * DV], BF16, kind="ExternalInput").ap()
    wo = nc.dram_tensor("wo", [HC * DV, D], BF16, kind="ExternalInput").ap()
    maskr = nc.dram_tensor("maskr", [128, NKC], F32, kind="ExternalInput").ap()
    out = nc.dram_tensor("out", [S, D], F32, kind="ExternalOutput").ap()

    with tile.TileContext(nc) as tc:
        with ExitStack() as ctx:
            const_pool = ctx.enter_context(tc.tile_pool(name="const", bufs=1))
            w_pool = ctx.enter_context(tc.tile_pool(name="weights", bufs=1))
            act_pool = ctx.enter_context(tc.tile_pool(name="acts", bufs=1))
            st_pool = ctx.enter_context(
                tc.tile_pool(name="stage", bufs=32))
            vt_pool = ctx.enter_context(tc.tile_pool(name="vtpool", bufs=1))
            # PSUM: pj(2, shared with bc) + sc(2x2) + mx(2) = 8 banks
            pj_pool = ctx.enter_context(
                tc.tile_pool(name="pjpsum", bufs=2, space="PSUM"))
            sc_pool = ctx.enter_context(
                tc.tile_pool(name="scpsum", bufs=2, space="PSUM"))
            mx_pool = ctx.enter_context(
                tc.tile_pool(name="mxpsum", bufs=1, space="PSUM"))
            exp_pool = ctx.enter_context(tc.tile_pool(name="exp", bufs=4))
            norm_pool = ctx.enter_context(tc.tile_pool(name="norm",
                                                       bufs=2 * NP))
            tmp_pool = ctx.enter_context(tc.tile_pool(name="tmp", bufs=2))
            out_pool = ctx.enter_context(tc.tile_pool(name="outsb", bufs=2))

            # weight tiles (DMAs issued in need-order below)
            wq_sb = w_pool.tile([128, NC_CHUNKS * 512], BF16, tag="wq")
            wk_sb = w_pool.tile([128, NC_CHUNKS * 512], BF16, tag="wk")
            wv_sb = w_pool.tile([128, NC_CHUNKS * 512], BF16, tag="wv")
            wo_sb = w_pool.tile([128, NP * 1024], BF16, tag="wo")

            mask_sb = const_pool.tile([128, NKC], F32)
            ones_sb = const_pool.tile([128, 64], BF16)

            # persistent activations
            qhT = [act_pool.tile([128, S], BF16, tag=f"qhT{p}", name=f"qhT{p}")
                   for p in range(NP)]
            khT = [act_pool.tile([128, S], BF16, tag=f"khT{p}",
                                 name=f"khT{p}") for p in range(NP)]
            vhs = [act_pool.tile([128, VW], BF16, tag=f"vh{t}", name=f"vh{t}")
                   for t in range(NKC)]

            # ---- issue order: q(qb0) path first ----
            for c in range(NC_CHUNKS):
                nc.sync.dma_start(wq_sb[:, c * 512:(c + 1) * 512],
                                  wq[c * 128:(c + 1) * 128, :])
            nc.sync.dma_start(mask_sb[:], maskr[:])
            nc.vector.memset(ones_sb[:], 1.0)
            for t in range(NKC):
                nc.vector.memset(vhs[t][:, :], 0.0)

            def stage_block(src, qb):
                stg = []
                for c in range(NC_CHUNKS):
                    t = st_pool.tile([128, 512], BF16, tag="stage",
                                     name=f"stg{c}")
                    nc.sync.dma_start(
                        t[:], src[c * 128:(c + 1) * 128,
                                  qb * 512:(qb + 1) * 512])
                    stg.append(t)
                return stg

            def proj_pair(stg, wsb, p, dst, qb):
                ps = pj_pool.tile([128, 512], F32, tag="pj", name="pps")
                for c in range(NC_CHUNKS):
                    nc.tensor.matmul(
                        ps[:],
                        lhsT=wsb[:, c * 512 + p * 128:c * 512 + (p + 1) * 128],
                        rhs=stg[c][:],
                        start=(c == 0), stop=(c == NC_CHUNKS - 1))
                nc.vector.tensor_copy(
                    dst[p][:, qb * 512:(qb + 1) * 512], ps[:])

            def vproj_granule(t):
                # v projection chunk t (with mask fold + ones col)
                ps = pj_pool.tile([128, 512], F32, tag="pj", name="vps")
                for c in range(NC_CHUNKS):
                    nc.tensor.matmul(
                        ps[:],
                        lhsT=vt_sb[c][:, t * 128:(t + 1) * 128],
                        rhs=wv_sb[:, c * 512:(c + 1) * 512],
                        start=(c == 0), stop=(c == NC_CHUNKS - 1))
                # masked copy into vh store (strided per head) + mask col
                dst_dv = vhs[t][:, 0:VW].rearrange(
                    "p (h x) -> p h x", x=128)[:, :, 0:DV]
                src_dv = ps[:].rearrange("p (h x) -> p h x", x=DV)
                nc.vector.tensor_scalar_mul(dst_dv, src_dv,
                                            mask_sb[:, t:t + 1])
                dst_m = vhs[t][:, 0:VW].rearrange(
                    "p (h x) -> p h x", x=128)[:, :, DV:DV + 1]
                src_m = ones_sb[:, 0:HC].rearrange("p (h x) -> p h x", x=1)
                nc.vector.tensor_scalar_mul(dst_m, src_m,
                                            mask_sb[:, t:t + 1])

            # q(qb=0) projection
            qstg = stage_block(qT, 0)
            for p in range(NP):
                proj_pair(qstg, wq_sb, p, qhT, 0)

            # k block 0 for all pairs upfront; kb1-3 JIT per pair below
            for c in range(NC_CHUNKS):
                nc.sync.dma_start(wk_sb[:, c * 512:(c + 1) * 512],
                                  wk[c * 128:(c + 1) * 128, :])
            kstg = {0: stage_block(kT, 0)}
            for p in range(NP):
                proj_pair(kstg[0], wk_sb, p, khT, 0)

            # v/o weights + vT staged token-major (early tokens first),
            # k stages for kb1-3 interleaved behind them
            for c in range(NC_CHUNKS):
                nc.sync.dma_start(wv_sb[:, c * 512:(c + 1) * 512],
                                  wv[c * 128:(c + 1) * 128, :])
            vt_sb = [vt_pool.tile([128, S], BF16, tag=f"vt{c}", name=f"vt{c}")
                     for c in range(NC_CHUNKS)]
            for tb in range(NQB):
                for c in range(NC_CHUNKS):
                    nc.sync.dma_start(
                        vt_sb[c][:, tb * 512:(tb + 1) * 512],
                        vT[c * 128:(c + 1) * 128, tb * 512:(tb + 1) * 512])
                if tb + 1 < NQB:
                    kstg[tb + 1] = stage_block(kT, tb + 1)
            for p in range(NP):
                nc.sync.dma_start(wo_sb[:, p * 1024:(p + 1) * 1024],
                                  wo[p * 128:(p + 1) * 128, :])

            # ---- attention + output projection ----
            # Software-pipelined over flat units u = (qb, p, kc): the
            # scores+exp issue runs LOOK units ahead of the mix issue so
            # ScalarE keeps exp-ing across pair boundaries while the PE
            # absorbs normalize/proj/Wo work in its slack.
            LOOK = 2
            units = [(qb, p, kc) for qb in range(NQB) for p in range(NP)
                     for kc in range(NKC)]
            pend = {}
            mix_tiles = {}
            stg_nxt = {}
            normT = {qb: [] for qb in range(NQB)}

            def issue_sc(u):
                qb, p, kc = u
                qsl = slice(qb * 512, (qb + 1) * 512)
                ksl = slice(kc * 128, (kc + 1) * 128)
                scP = sc_pool.tile([128, 1024], F32, tag="sc")
                # 64x128 PE row tiling: both heads co-stream
                nc.tensor.matmul(
                    scP[:, 0:512],
                    lhsT=khT[p][0:64, ksl], rhs=qhT[p][0:64, qsl],
                    start=True, stop=True, tile_position=(0, 0))
                nc.tensor.matmul(
                    scP[:, 512:1024],
                    lhsT=khT[p][64:128, ksl], rhs=qhT[p][64:128, qsl],
                    start=True, stop=True, tile_position=(64, 0))
                exP = exp_pool.tile([128, 1024], BF16, tag="exp")
                nc.scalar.activation(exP[:], scP[:], EXP)
                pend[u] = exP

            def issue_mix(u):
                qb, p, kc = u
                h0, h1 = 2 * p, 2 * p + 1
                if kc == 0:
                    mix_tiles[(qb, p)] = mx_pool.tile(
                        [128, 1024], F32, tag="mx", name="mixPR")
                mixPR = mix_tiles[(qb, p)]
                exP = pend.pop(u)
                va = vhs[kc]
                st = (kc == 0)
                sp = (kc == NKC - 1)
                nc.tensor.matmul(
                    mixPR[:, 0:512], lhsT=va[:, h0 * 128:(h0 + 1) * 128],
                    rhs=exP[:, 0:512], start=st, stop=sp)
                nc.tensor.matmul(
                    mixPR[:, 512:1024], lhsT=va[:, h1 * 128:(h1 + 1) * 128],
                    rhs=exP[:, 512:1024], start=st, stop=sp)

            def normalize(qb, p):
                # evac mix PSUM to SBUF immediately (frees the banks fast)
                mixPR = mix_tiles.pop((qb, p))
                mloc = tmp_pool.tile([128, 1024], F32, tag="mloc")
                nc.vector.tensor_copy(mloc[:], mixPR[:])
                nt = norm_pool.tile([128, 512], BF16, tag="norm")
                normT[qb].append(nt)

                def finish():
                    # Z row -> partition 0 (DMA hop; partition_broadcast
                    # reads physical partition 0), bcast across partitions
                    # on idle GpSimd, then reciprocal + scale on DVE
                    zr0 = tmp_pool.tile([1, 1024], F32, tag="zr0")
                    nc.sync.dma_start(zr0[:], mloc[64:65, :])
                    zb = tmp_pool.tile([64, 1024], F32, tag="zb")
                    nc.gpsimd.partition_broadcast(zb[:], zr0[:])
                    rec = zb
                    nc.vector.reciprocal_approx_fast(rec[:], zb[:])
                    nc.vector.tensor_mul(nt[0:64, :], mloc[0:64, 0:512],
                                         rec[:, 0:512])
                    sh1 = tmp_pool.tile([64, 512], BF16, tag="sh1")
                    nc.vector.tensor_mul(sh1[:], mloc[0:64, 512:1024],
                                         rec[:, 512:1024])
                    nc.sync.dma_start(nt[64:128, :], sh1[:])
                return finish

            deferred = []

            def wo_piece(qb, tt, dh):
                def run():
                    wps = pj_pool.tile([128, 512], F32, tag="pj",
                                       name="wps")
                    for p in range(NP):
                        nc.tensor.matmul(
                            wps[:],
                            lhsT=normT[qb][p][:, tt * 128:(tt + 1) * 128],
                            rhs=wo_sb[:, p * 1024 + dh * 512:
                                      p * 1024 + (dh + 1) * 512],
                            start=(p == 0), stop=(p == NP - 1))
                    osb = out_pool.tile([128, 512], F32, tag="osb",
                                        name="osb")
                    nc.vector.tensor_copy(osb[:], wps[:])
                    nc.sync.dma_start(
                        out[qb * 512 + tt * 128:qb * 512 + (tt + 1) * 128,
                            dh * 512:(dh + 1) * 512], osb[:])
                return run

            def proj_piece(qb, p, cs):
                def run():
                    stg = stg_nxt[qb]
                    key = ("pps", qb, p)
                    if cs == 0:
                        mix_tiles[key] = pj_pool.tile(
                            [128, 512], F32, tag="pj", name="pps")
                    ps = mix_tiles[key]
                    for c in (cs, cs + 1):
                        nc.tensor.matmul(
                            ps[:],
                            lhsT=wq_sb[:, c * 512 + p * 128:
                                       c * 512 + (p + 1) * 128],
                            rhs=stg[c][:],
                            start=(c == 0), stop=(c == NC_CHUNKS - 1))
                    if cs + 2 == NC_CHUNKS:
                        nc.vector.tensor_copy(
                            qhT[p][:, qb * 512:(qb + 1) * 512], ps[:])
                        del mix_tiles[key]
                return run

            kdone = {(p, 0) for p in range(NP)}
            for i, u in enumerate(units):
                if i == 0:
                    for j in range(LOOK):
                        issue_sc(units[j])
                if u[0] == 0 and u[1] == 0:
                    vproj_granule(u[2])
                issue_mix(u)
                if i + LOOK < len(units):
                    nxt = units[i + LOOK]
                    if nxt[0] == 0 and nxt[2] % 4 == 0:
                        kk = (nxt[1], nxt[2] // 4)
                        if kk not in kdone:
                            kdone.add(kk)
                            proj_pair(kstg[kk[1]], wk_sb, kk[0], khT, kk[1])
                    issue_sc(nxt)
                if deferred:
                    deferred.pop(0)()
                qb, p, kc = u
                if p == 0 and kc == 14 and qb + 1 < NQB:
                    stg_nxt[qb + 1] = stage_block(qT, qb + 1)
                if kc == NKC - 1:
                    deferred.append(normalize(qb, p))
                    if qb + 1 < NQB:
                        deferred.extend(
                            proj_piece(qb + 1, p, cs)
                            for cs in range(0, NC_CHUNKS, 2))
                    if p == NP - 1:
                        deferred.extend(wo_piece(qb, tt, dh)
                                        for tt in range(4) for dh in range(2))
            while deferred:
                deferred.pop(0)()

    nc.compile()
    return nc


def _get_nc():
    if "nc" not in _COMPILED:
        _COMPILED["nc"] = _build_nc()
    return _COMPILED["nc"]


def _shard_inputs(q, k, v, mask, Wq, Wk, Wv, Wo):
    """Build the per-core input maps (host-side layout prep)."""
    import ml_dtypes

    bf16 = ml_dtypes.bfloat16
    in_maps = []
    maskf = np.asarray(mask).astype(np.float32)
    q = np.asarray(q, np.float32)
    k = np.asarray(k, np.float32)
    v = np.asarray(v, np.float32)
    Wq = np.asarray(Wq, np.float32)
    Wk = np.asarray(Wk, np.float32)
    Wv = np.asarray(Wv, np.float32)
    Wo = np.asarray(Wo, np.float32)
    scale = np.float32(1.0 / np.sqrt(DK))
    for c in range(NCORES):
        b, hg = c // 2, c % 2
        hs = hg * HC
        m = {
            "qT": np.ascontiguousarray(q[b].T).astype(bf16),
            "kT": np.ascontiguousarray(k[b].T).astype(bf16),
            "vT": np.ascontiguousarray(v[b].T).astype(bf16),
            # head-major col blocks; fold 1/sqrt(dk) into Wq
            "wq": np.ascontiguousarray(
                Wq[hs:hs + HC].transpose(1, 0, 2).reshape(D, HC * DK) * scale
            ).astype(bf16),
            "wk": np.ascontiguousarray(
                Wk[hs:hs + HC].transpose(1, 0, 2).reshape(D, HC * DK)
            ).astype(bf16),
            "wv": np.ascontiguousarray(
                Wv[hs:hs + HC].transpose(1, 0, 2).reshape(D, HC * DV)
            ).astype(bf16),
            "wo": np.ascontiguousarray(Wo[hs * DV:(hs + HC) * DV]).astype(bf16),
            "maskr": np.ascontiguousarray(
                maskf[b].reshape(NKC, 128).T).astype(np.float32),
        }
        in_maps.append(m)
    return in_maps


def kernel(q, k, v, mask, Wq, Wk, Wv, Wo, _trace=False):
    from concourse.bass_utils import run_bass_kernel_spmd

    nc = _get_nc()
    in_maps = _shard_inputs(q, k, v, mask, Wq, Wk, Wv, Wo)
    res = run_bass_kernel_spmd(nc, in_maps, list(range(NCORES)),
                               trace=_trace)
    out = np.zeros((B, S, D), np.float32)
    for c in range(NCORES):
        out[c // 2] += res.results[c]["out"]
    if _trace:
        _COMPILED["last_result"] = res
    return out


# revision 34
# speedup vs baseline: 1.0503x; 1.0156x over previous
"""Multi-head attention (B=4, S=2048, D=1024, H=16, dk=dv=64) on 8 TRN2 cores.

Sharding: core c = 2*b + hg handles batch b = c//2 and heads
[hg*8, hg*8+8). Each core computes a partial output
(its 8 heads' contribution through Wo); the host adds the two partials
per batch.

Per-core device pipeline (matmul inputs bf16, PSUM accumulation fp32,
softmax sums/reciprocal fp32):
  1. q(qb=0) projection first (shortest path to attention), then khT
     projections (pair layout: h0 dk on partitions 0-63, h1 on 64-127),
     then vh projection per key-chunk as [128, 8*128] bf16 with a
     mask/ones column appended per head (masked keys zeroed; cols 65-127
     zero). q(qb+1) projections are interleaved into attention qb.
  2. scores^T per head pair via 64x128 PE row tiling: per key-chunk one
     [128, 1024] PSUM tile holds h0 scores (cols 0-511, tile (0,0)) and
     h1 scores (cols 512-1023, tile (64,0)); the two matmuls co-stream
     in the PE array (separate PSUM banks).
  3. exp on ScalarE PSUM->SBUF bf16, one [128, 1024] ACTIVATE per chunk.
  4. mix^T + softmax sums in one matmul: lhsT = vh block [128 keys,
     128] (col 64 = mask/ones), rhs = exp half [128, 512]; PSUM
     accumulation over the 16 chunks (mixP for h0, mixR for h1).
  5. normalize: Z row (partition 64) -> bf16 SBUF, K=1 PE matmul
     broadcasts it to partitions 0-63, reciprocal_approx_fast at base
     partition 0 (custom-DVE ops misbehave at base partition 64),
     multiply mix rows by 1/Z (bf16 out). h1's normalized tile is
     DMA-shifted to partitions 64-127 so each pair's mix^T is one
     [128, 512] tile (e on partitions).
  6. out += mixT_norm.T @ Wo: dense K=128 bf16 matmuls accumulating over
     the 4 pairs; DVE evac fp32 -> DMA to HBM.
"""

import numpy as np

B, S, D = 4, 2048, 1024
H, DK, DV = 16, 64, 64
HC = 8          # heads per core
NP = HC // 2    # head pairs per core
NCORES = 8
NC_CHUNKS = D // 128    # 8 contraction chunks over D
NKC = S // 128          # 16 key chunks
NQB = S // 512          # 4 query blocks
VW = HC * 128           # vh storage: 128 cols per head (dv | mask | zeros)

_COMPILED = {}


def _build_nc():
    import concourse.tile as tile
    from concourse import bacc, mybir
    from contextlib import ExitStack

    F32 = mybir.dt.float32
    BF16 = mybir.dt.bfloat16
    EXP = mybir.ActivationFunctionType.Exp

    nc = bacc.Bacc("TRN2", target_bir_lowering=False, debug=False,
                   num_devices=NCORES)

    qT = nc.dram_tensor("qT", [D, S], BF16, kind="ExternalInput").ap()
    kT = nc.dram_tensor("kT", [D, S], BF16, kind="ExternalInput").ap()
    vT = nc.dram_tensor("vT", [D, S], BF16, kind="ExternalInput").ap()
    wq = nc.dram_tensor("wq", [D, HC * DK], BF16, kind="ExternalInput").ap()
    wk = nc.dram_tensor("wk", [D, HC * DK], BF16, kind="ExternalInput").ap()
    wv = nc.dram_tensor("wv", [D, HC * DV], BF16, kind="ExternalInput").ap()
    wo = nc.dram_tensor("wo", [HC * DV, D], BF16, kind="ExternalInput").ap()
    maskr = nc.dram_tensor("maskr", [128, NKC], F32, kind="ExternalInput").ap()
    out = nc.dram_tensor("out", [S, D], F32, kind="ExternalOutput").ap()

    with tile.TileContext(nc) as tc:
        with ExitStack() as ctx:
            const_pool = ctx.enter_context(tc.tile_pool(name="const", bufs=1))
            w_pool = ctx.enter_context(tc.tile_pool(name="weights", bufs=1))
            act_pool = ctx.enter_context(tc.tile_pool(name="acts", bufs=1))
            st_pool = ctx.enter_context(
                tc.tile_pool(name="stage", bufs=32))
            vt_pool = ctx.enter_context(tc.tile_pool(name="vtpool", bufs=1))
            # PSUM: pj(2, shared with bc) + sc(2x2) + mx(2) = 8 banks
            pj_pool = ctx.enter_context(
                tc.tile_pool(name="pjpsum", bufs=2, space="PSUM"))
            sc_pool = ctx.enter_context(
                tc.tile_pool(name="scpsum", bufs=2, space="PSUM"))
            mx_pool = ctx.enter_context(
                tc.tile_pool(name="mxpsum", bufs=1, space="PSUM"))
            exp_pool = ctx.enter_context(tc.tile_pool(name="exp", bufs=4))
            norm_pool = ctx.enter_context(tc.tile_pool(name="norm",
                                                       bufs=2 * NP))
            tmp_pool = ctx.enter_context(tc.tile_pool(name="tmp", bufs=2))
            out_pool = ctx.enter_context(tc.tile_pool(name="outsb", bufs=2))

            # weight tiles (DMAs issued in need-order below)
            wq_sb = w_pool.tile([128, NC_CHUNKS * 512], BF16, tag="wq")
            wk_sb = w_pool.tile([128, NC_CHUNKS * 512], BF16, tag="wk")
            wv_sb = w_pool.tile([128, NC_CHUNKS * 512], BF16, tag="wv")
            wo_sb = w_pool.tile([128, NP * 1024], BF16, tag="wo")

            mask_sb = const_pool.tile([128, NKC], F32)
            ones_sb = const_pool.tile([128, 64], BF16)

            # persistent activations
            qhT = [act_pool.tile([128, S], BF16, tag=f"qhT{p}", name=f"qhT{p}")
                   for p in range(NP)]
            khT = [act_pool.tile([128, S], BF16, tag=f"khT{p}",
                                 name=f"khT{p}") for p in range(NP)]
            vhs = [act_pool.tile([128, VW], BF16, tag=f"vh{t}", name=f"vh{t}")
                   for t in range(NKC)]

            # ---- issue order: q(qb0) path first ----
            for c in range(NC_CHUNKS):
                nc.sync.dma_start(wq_sb[:, c * 512:(c + 1) * 512],
                                  wq[c * 128:(c + 1) * 128, :])
            nc.sync.dma_start(mask_sb[:], maskr[:])
            nc.vector.memset(ones_sb[:], 1.0)
            for t in range(NKC):
                nc.vector.memset(vhs[t][:, :], 0.0)

            def stage_block(src, qb):
                stg = []
                for c in range(NC_CHUNKS):
                    t = st_pool.tile([128, 512], BF16, tag="stage",
                                     name=f"stg{c}")
                    nc.sync.dma_start(
                        t[:], src[c * 128:(c + 1) * 128,
                                  qb * 512:(qb + 1) * 512])
                    stg.append(t)
                return stg

            def proj_pair(stg, wsb, p, dst, qb):
                ps = pj_pool.tile([128, 512], F32, tag="pj", name="pps")
                for c in range(NC_CHUNKS):
                    nc.tensor.matmul(
                        ps[:],
                        lhsT=wsb[:, c * 512 + p * 128:c * 512 + (p + 1) * 128],
                        rhs=stg[c][:],
                        start=(c == 0), stop=(c == NC_CHUNKS - 1))
                nc.vector.tensor_copy(
                    dst[p][:, qb * 512:(qb + 1) * 512], ps[:])

            def vproj_granule(t):
                # v projection chunk t (with mask fold + ones col)
                ps = pj_pool.tile([128, 512], F32, tag="pj", name="vps")
                for c in range(NC_CHUNKS):
                    nc.tensor.matmul(
                        ps[:],
                        lhsT=vt_sb[c][:, t * 128:(t + 1) * 128],
                        rhs=wv_sb[:, c * 512:(c + 1) * 512],
                        start=(c == 0), stop=(c == NC_CHUNKS - 1))
                # masked copy into vh store (strided per head) + mask col
                dst_dv = vhs[t][:, 0:VW].rearrange(
                    "p (h x) -> p h x", x=128)[:, :, 0:DV]
                src_dv = ps[:].rearrange("p (h x) -> p h x", x=DV)
                nc.vector.tensor_scalar_mul(dst_dv, src_dv,
                                            mask_sb[:, t:t + 1])
                dst_m = vhs[t][:, 0:VW].rearrange(
                    "p (h x) -> p h x", x=128)[:, :, DV:DV + 1]
                src_m = ones_sb[:, 0:HC].rearrange("p (h x) -> p h x", x=1)
                nc.vector.tensor_scalar_mul(dst_m, src_m,
                                            mask_sb[:, t:t + 1])

            # q(qb=0) projection
            qstg = stage_block(qT, 0)
            for p in range(NP):
                proj_pair(qstg, wq_sb, p, qhT, 0)

            # k block 0 for all pairs upfront; kb1-3 JIT per pair below
            for c in range(NC_CHUNKS):
                nc.sync.dma_start(wk_sb[:, c * 512:(c + 1) * 512],
                                  wk[c * 128:(c + 1) * 128, :])
            kstg = {0: stage_block(kT, 0)}
            for p in range(NP):
                proj_pair(kstg[0], wk_sb, p, khT, 0)

            # v/o weights + vT staged token-major (early tokens first),
            # k stages for kb1-3 interleaved behind them
            for c in range(NC_CHUNKS):
                nc.sync.dma_start(wv_sb[:, c * 512:(c + 1) * 512],
                                  wv[c * 128:(c + 1) * 128, :])
            vt_sb = [vt_pool.tile([128, S], BF16, tag=f"vt{c}", name=f"vt{c}")
                     for c in range(NC_CHUNKS)]
            for tb in range(NQB):
                for c in range(NC_CHUNKS):
                    nc.sync.dma_start(
                        vt_sb[c][:, tb * 512:(tb + 1) * 512],
                        vT[c * 128:(c + 1) * 128, tb * 512:(tb + 1) * 512])
                if tb + 1 < NQB:
                    kstg[tb + 1] = stage_block(kT, tb + 1)
            for p in range(NP):
                nc.sync.dma_start(wo_sb[:, p * 1024:(p + 1) * 1024],
                                  wo[p * 128:(p + 1) * 128, :])

            # ---- attention + output projection ----
            # Software-pipelined over flat units u = (qb, p, kc): the
            # scores+exp issue runs LOOK units ahead of the mix issue so
            # ScalarE keeps exp-ing across pair boundaries while the PE
            # absorbs normalize/proj/Wo work in its slack.
            LOOK = 2
            units = [(qb, p, kc) for qb in range(NQB) for p in range(NP)
                     for kc in range(NKC)]
            pend = {}
            mix_tiles = {}
            stg_nxt = {}
            normT = {qb: [] for qb in range(NQB)}

            def issue_sc(u):
                qb, p, kc = u
                qsl = slice(qb * 512, (qb + 1) * 512)
                ksl = slice(kc * 128, (kc + 1) * 128)
                scP = sc_pool.tile([128, 1024], F32, tag="sc")
                # 64x128 PE row tiling: both heads co-stream
                nc.tensor.matmul(
                    scP[:, 0:512],
                    lhsT=khT[p][0:64, ksl], rhs=qhT[p][0:64, qsl],
                    start=True, stop=True, tile_position=(0, 0))
                nc.tensor.matmul(
                    scP[:, 512:1024],
                    lhsT=khT[p][64:128, ksl], rhs=qhT[p][64:128, qsl],
                    start=True, stop=True, tile_position=(64, 0))
                exP = exp_pool.tile([128, 1024], BF16, tag="exp")
                nc.scalar.activation(exP[:], scP[:], EXP)
                pend[u] = exP

            def issue_mix(u):
                qb, p, kc = u
                h0, h1 = 2 * p, 2 * p + 1
                if kc == 0:
                    mix_tiles[(qb, p)] = mx_pool.tile(
                        [128, 1024], F32, tag="mx", name="mixPR")
                mixPR = mix_tiles[(qb, p)]
                exP = pend.pop(u)
                va = vhs[kc]
                st = (kc == 0)
                sp = (kc == NKC - 1)
                nc.tensor.matmul(
                    mixPR[:, 0:512], lhsT=va[:, h0 * 128:(h0 + 1) * 128],
                    rhs=exP[:, 0:512], start=st, stop=sp)
                nc.tensor.matmul(
                    mixPR[:, 512:1024], lhsT=va[:, h1 * 128:(h1 + 1) * 128],
                    rhs=exP[:, 512:1024], start=st, stop=sp)

            def normalize(qb, p):
                # evac mix PSUM to SBUF immediately (frees the banks fast)
                mixPR = mix_tiles.pop((qb, p))
                mloc = tmp_pool.tile([128, 1024], F32, tag="mloc")
                nc.vector.tensor_copy(mloc[:], mixPR[:])
                nt = norm_pool.tile([128, 512], BF16, tag="norm")
                normT[qb].append(nt)

                def finish():
                    # Z row -> partition 0 (DMA hop; partition_broadcast
                    # reads physical partition 0), bcast across partitions
                    # on idle GpSimd, then reciprocal + scale on DVE
                    zr0 = tmp_pool.tile([1, 1024], F32, tag="zr0")
                    nc.sync.dma_start(zr0[:], mloc[64:65, :])
                    zb = tmp_pool.tile([64, 1024], F32, tag="zb")
                    nc.gpsimd.partition_broadcast(zb[:], zr0[:])
                    rec = zb
                    nc.vector.reciprocal_approx_fast(rec[:], zb[:])
                    nc.vector.tensor_mul(nt[0:64, :], mloc[0:64, 0:512],
                                         rec[:, 0:512])
                    sh1 = tmp_pool.tile([64, 512], BF16, tag="sh1")
                    nc.vector.tensor_mul(sh1[:], mloc[0:64, 512:1024],
                                         rec[:, 512:1024])
                    nc.sync.dma_start(nt[64:128, :], sh1[:])
                return finish

            deferred = []

            def wo_piece(qb, tt, dh, half):
                def run():
                    key = ("wops", qb, tt, dh)
                    if half == 0:
                        mix_tiles[key] = pj_pool.tile(
                            [128, 512], F32, tag="pj", name="wps")
                    wps = mix_tiles[key]
                    for p in (0, 1) if half == 0 else (2, 3):
                        nc.tensor.matmul(
                            wps[:],
                            lhsT=normT[qb][p][:, tt * 128:(tt + 1) * 128],
                            rhs=wo_sb[:, p * 1024 + dh * 512:
                                      p * 1024 + (dh + 1) * 512],
                            start=(p == 0), stop=(p == NP - 1))
                    if half == 1:
                        del mix_tiles[key]
                        osb = out_pool.tile([128, 512], F32, tag="osb",
                                            name="osb")
                        nc.vector.tensor_copy(osb[:], wps[:])
                        nc.sync.dma_start(
                            out[qb * 512 + tt * 128:
                                qb * 512 + (tt + 1) * 128,
                                dh * 512:(dh + 1) * 512], osb[:])
                return run

            def proj_piece(qb, p, cs):
                def run():
                    stg = stg_nxt[qb]
                    key = ("pps", qb, p)
                    if cs == 0:
                        mix_tiles[key] = pj_pool.tile(
                            [128, 512], F32, tag="pj", name="pps")
                    ps = mix_tiles[key]
                    for c in (cs, cs + 1):
                        nc.tensor.matmul(
                            ps[:],
                            lhsT=wq_sb[:, c * 512 + p * 128:
                                       c * 512 + (p + 1) * 128],
                            rhs=stg[c][:],
                            start=(c == 0), stop=(c == NC_CHUNKS - 1))
                    if cs + 2 == NC_CHUNKS:
                        nc.vector.tensor_copy(
                            qhT[p][:, qb * 512:(qb + 1) * 512], ps[:])
                        del mix_tiles[key]
                return run

            kdone = {(p, 0) for p in range(NP)}
            for i, u in enumerate(units):
                if i == 0:
                    for j in range(LOOK):
                        issue_sc(units[j])
                if u[0] == 0 and u[1] == 0:
                    vproj_granule(u[2])
                issue_mix(u)
                if i + LOOK < len(units):
                    nxt = units[i + LOOK]
                    if nxt[0] == 0 and nxt[2] % 4 == 0:
                        kk = (nxt[1], nxt[2] // 4)
                        if kk not in kdone:
                            kdone.add(kk)
                            proj_pair(kstg[kk[1]], wk_sb, kk[0], khT, kk[1])
                    issue_sc(nxt)
                if deferred:
                    deferred.pop(0)()
                qb, p, kc = u
                if p == 0 and kc == 14 and qb + 1 < NQB:
                    stg_nxt[qb + 1] = stage_block(qT, qb + 1)
                if kc == NKC - 1:
                    deferred.append(normalize(qb, p))
                    if qb + 1 < NQB:
                        deferred.extend(
                            proj_piece(qb + 1, p, cs)
                            for cs in range(0, NC_CHUNKS, 2))
                    if p == NP - 1:
                        deferred.extend(wo_piece(qb, tt, dh, half)
                                        for tt in range(4) for dh in range(2)
                                        for half in range(2))
            while deferred:
                deferred.pop(0)()

    nc.compile()
    return nc


def _get_nc():
    if "nc" not in _COMPILED:
        _COMPILED["nc"] = _build_nc()
    return _COMPILED["nc"]


def _shard_inputs(q, k, v, mask, Wq, Wk, Wv, Wo):
    """Build the per-core input maps (host-side layout prep)."""
    import ml_dtypes

    bf16 = ml_dtypes.bfloat16
    in_maps = []
    maskf = np.asarray(mask).astype(np.float32)
    q = np.asarray(q, np.float32)
    k = np.asarray(k, np.float32)
    v = np.asarray(v, np.float32)
    Wq = np.asarray(Wq, np.float32)
    Wk = np.asarray(Wk, np.float32)
    Wv = np.asarray(Wv, np.float32)
    Wo = np.asarray(Wo, np.float32)
    scale = np.float32(1.0 / np.sqrt(DK))
    for c in range(NCORES):
        b, hg = c // 2, c % 2
        hs = hg * HC
        m = {
            "qT": np.ascontiguousarray(q[b].T).astype(bf16),
            "kT": np.ascontiguousarray(k[b].T).astype(bf16),
            "vT": np.ascontiguousarray(v[b].T).astype(bf16),
            # head-major col blocks; fold 1/sqrt(dk) into Wq
            "wq": np.ascontiguousarray(
                Wq[hs:hs + HC].transpose(1, 0, 2).reshape(D, HC * DK) * scale
            ).astype(bf16),
            "wk": np.ascontiguousarray(
                Wk[hs:hs + HC].transpose(1, 0, 2).reshape(D, HC * DK)
            ).astype(bf16),
            "wv": np.ascontiguousarray(
                Wv[hs:hs + HC].transpose(1, 0, 2).reshape(D, HC * DV)
            ).astype(bf16),
            "wo": np.ascontiguousarray(Wo[hs * DV:(hs + HC) * DV]).astype(bf16),
            "maskr": np.ascontiguousarray(
                maskf[b].reshape(NKC, 128).T).astype(np.float32),
        }
        in_maps.append(m)
    return in_maps


def kernel(q, k, v, mask, Wq, Wk, Wv, Wo, _trace=False):
    from concourse.bass_utils import run_bass_kernel_spmd

    nc = _get_nc()
    in_maps = _shard_inputs(q, k, v, mask, Wq, Wk, Wv, Wo)
    res = run_bass_kernel_spmd(nc, in_maps, list(range(NCORES)),
                               trace=_trace)
    out = np.zeros((B, S, D), np.float32)
    for c in range(NCORES):
        out[c // 2] += res.results[c]["out"]
    if _trace:
        _COMPILED["last_result"] = res
    return out


# revision 41
# speedup vs baseline: 1.0553x; 1.0047x over previous
"""Multi-head attention (B=4, S=2048, D=1024, H=16, dk=dv=64) on 8 TRN2 cores.

Sharding: core c = 2*b + hg handles batch b = c//2 and heads
[hg*8, hg*8+8). Each core computes a partial output
(its 8 heads' contribution through Wo); the host adds the two partials
per batch.

Per-core device pipeline (matmul inputs bf16, PSUM accumulation fp32,
softmax sums/reciprocal fp32):
  1. q(qb=0) projection first (shortest path to attention), then khT
     projections (pair layout: h0 dk on partitions 0-63, h1 on 64-127),
     then vh projection per key-chunk as [128, 8*128] bf16 with a
     mask/ones column appended per head (masked keys zeroed; cols 65-127
     zero). q(qb+1) projections are interleaved into attention qb.
  2. scores^T per head pair via 64x128 PE row tiling: per key-chunk one
     [128, 1024] PSUM tile holds h0 scores (cols 0-511, tile (0,0)) and
     h1 scores (cols 512-1023, tile (64,0)); the two matmuls co-stream
     in the PE array (separate PSUM banks).
  3. exp on ScalarE PSUM->SBUF bf16, one [128, 1024] ACTIVATE per chunk.
  4. mix^T + softmax sums in one matmul: lhsT = vh block [128 keys,
     128] (col 64 = mask/ones), rhs = exp half [128, 512]; PSUM
     accumulation over the 16 chunks (mixP for h0, mixR for h1).
  5. normalize: Z row (partition 64) -> bf16 SBUF, K=1 PE matmul
     broadcasts it to partitions 0-63, reciprocal_approx_fast at base
     partition 0 (custom-DVE ops misbehave at base partition 64),
     multiply mix rows by 1/Z (bf16 out). h1's normalized tile is
     DMA-shifted to partitions 64-127 so each pair's mix^T is one
     [128, 512] tile (e on partitions).
  6. out += mixT_norm.T @ Wo: dense K=128 bf16 matmuls accumulating over
     the 4 pairs; DVE evac fp32 -> DMA to HBM.
"""

import numpy as np

B, S, D = 4, 2048, 1024
H, DK, DV = 16, 64, 64
HC = 8          # heads per core
NP = HC // 2    # head pairs per core
NCORES = 8
NC_CHUNKS = D // 128    # 8 contraction chunks over D
NKC = S // 128          # 16 key chunks
NQB = S // 512          # 4 query blocks
VW = HC * 128           # vh storage: 128 cols per head (dv | mask | zeros)

_COMPILED = {}


def _build_nc():
    import concourse.tile as tile
    from concourse import bacc, mybir
    from contextlib import ExitStack

    F32 = mybir.dt.float32
    BF16 = mybir.dt.bfloat16
    EXP = mybir.ActivationFunctionType.Exp

    nc = bacc.Bacc("TRN2", target_bir_lowering=False, debug=False,
                   num_devices=NCORES)

    qT = nc.dram_tensor("qT", [D, S], BF16, kind="ExternalInput").ap()
    kT = nc.dram_tensor("kT", [D, S], BF16, kind="ExternalInput").ap()
    vT = nc.dram_tensor("vT", [D, S], BF16, kind="ExternalInput").ap()
    wq = nc.dram_tensor("wq", [D, HC * DK], BF16, kind="ExternalInput").ap()
    wk = nc.dram_tensor("wk", [D, HC * DK], BF16, kind="ExternalInput").ap()
    wv = nc.dram_tensor("wv", [D, HC * DV], BF16, kind="ExternalInput").ap()
    wo = nc.dram_tensor("wo", [HC * DV, D], BF16, kind="ExternalInput").ap()
    maskr = nc.dram_tensor("maskr", [128, NKC], F32, kind="ExternalInput").ap()
    out = nc.dram_tensor("out", [S, D], F32, kind="ExternalOutput").ap()

    with tile.TileContext(nc) as tc:
        with ExitStack() as ctx:
            const_pool = ctx.enter_context(tc.tile_pool(name="const", bufs=1))
            w_pool = ctx.enter_context(tc.tile_pool(name="weights", bufs=1))
            act_pool = ctx.enter_context(tc.tile_pool(name="acts", bufs=1))
            st_pool = ctx.enter_context(
                tc.tile_pool(name="stage", bufs=32))
            vt_pool = ctx.enter_context(tc.tile_pool(name="vtpool", bufs=1))
            # PSUM: pj(2, shared with bc) + sc(2x2) + mx(2) = 8 banks
            pj_pool = ctx.enter_context(
                tc.tile_pool(name="pjpsum", bufs=2, space="PSUM"))
            sc_pool = ctx.enter_context(
                tc.tile_pool(name="scpsum", bufs=2, space="PSUM"))
            mx_pool = ctx.enter_context(
                tc.tile_pool(name="mxpsum", bufs=1, space="PSUM"))
            exp_pool = ctx.enter_context(tc.tile_pool(name="exp", bufs=4))
            norm_pool = ctx.enter_context(tc.tile_pool(name="norm",
                                                       bufs=2 * NP))
            tmp_pool = ctx.enter_context(tc.tile_pool(name="tmp", bufs=2))
            out_pool = ctx.enter_context(tc.tile_pool(name="outsb", bufs=2))

            # weight tiles (DMAs issued in need-order below)
            wq_sb = w_pool.tile([128, NC_CHUNKS * 512], BF16, tag="wq")
            wk_sb = w_pool.tile([128, NC_CHUNKS * 512], BF16, tag="wk")
            wv_sb = w_pool.tile([128, NC_CHUNKS * 512], BF16, tag="wv")
            wo_sb = w_pool.tile([128, NP * 1024], BF16, tag="wo")

            mask_sb = const_pool.tile([128, NKC], F32)
            ones_sb = const_pool.tile([128, 64], BF16)
            e65_sb = const_pool.tile([128, DV + 1], BF16)

            # persistent activations
            qhT = [act_pool.tile([128, S], BF16, tag=f"qhT{p}", name=f"qhT{p}")
                   for p in range(NP)]
            khT = [act_pool.tile([128, S], BF16, tag=f"khT{p}",
                                 name=f"khT{p}") for p in range(NP)]
            vhs = [act_pool.tile([128, VW], BF16, tag=f"vh{t}", name=f"vh{t}")
                   for t in range(NKC)]

            # ---- issue order: q(qb0) path first ----
            for c in range(NC_CHUNKS):
                nc.sync.dma_start(wq_sb[:, c * 512:(c + 1) * 512],
                                  wq[c * 128:(c + 1) * 128, :])
            nc.sync.dma_start(mask_sb[:], maskr[:])
            nc.vector.memset(ones_sb[:], 1.0)
            nc.vector.memset(e65_sb[64:65, :], 1.0)
            for t in range(NKC):
                nc.vector.memset(vhs[t][:, :], 0.0)

            def stage_block(src, qb):
                stg = []
                for c in range(NC_CHUNKS):
                    t = st_pool.tile([128, 512], BF16, tag="stage",
                                     name=f"stg{c}")
                    nc.sync.dma_start(
                        t[:], src[c * 128:(c + 1) * 128,
                                  qb * 512:(qb + 1) * 512])
                    stg.append(t)
                return stg

            def proj_pair(stg, wsb, p, dst, qb):
                ps = pj_pool.tile([128, 512], F32, tag="pj", name="pps")
                for c in range(NC_CHUNKS):
                    nc.tensor.matmul(
                        ps[:],
                        lhsT=wsb[:, c * 512 + p * 128:c * 512 + (p + 1) * 128],
                        rhs=stg[c][:],
                        start=(c == 0), stop=(c == NC_CHUNKS - 1))
                nc.vector.tensor_copy(
                    dst[p][:, qb * 512:(qb + 1) * 512], ps[:])

            def vproj_granule(t):
                # v projection chunk t (with mask fold + ones col)
                ps = pj_pool.tile([128, 512], F32, tag="pj", name="vps")
                for c in range(NC_CHUNKS):
                    nc.tensor.matmul(
                        ps[:],
                        lhsT=vt_sb[c][:, t * 128:(t + 1) * 128],
                        rhs=wv_sb[:, c * 512:(c + 1) * 512],
                        start=(c == 0), stop=(c == NC_CHUNKS - 1))
                # masked copy into vh store (strided per head) + mask col
                dst_dv = vhs[t][:, 0:VW].rearrange(
                    "p (h x) -> p h x", x=128)[:, :, 0:DV]
                src_dv = ps[:].rearrange("p (h x) -> p h x", x=DV)
                nc.vector.tensor_scalar_mul(dst_dv, src_dv,
                                            mask_sb[:, t:t + 1])
                dst_m = vhs[t][:, 0:VW].rearrange(
                    "p (h x) -> p h x", x=128)[:, :, DV:DV + 1]
                src_m = ones_sb[:, 0:HC].rearrange("p (h x) -> p h x", x=1)
                nc.vector.tensor_scalar_mul(dst_m, src_m,
                                            mask_sb[:, t:t + 1])

            # q(qb=0) / k block 0 projections: pair 0 first so the first
            # score pair (issued right below) reaches ScalarE early
            qstg = stage_block(qT, 0)
            proj_pair(qstg, wq_sb, 0, qhT, 0)
            for c in range(NC_CHUNKS):
                nc.sync.dma_start(wk_sb[:, c * 512:(c + 1) * 512],
                                  wk[c * 128:(c + 1) * 128, :])
            kstg = {0: stage_block(kT, 0)}
            proj_pair(kstg[0], wk_sb, 0, khT, 0)

            # v/o weights + vT staged token-major (early tokens first),
            # k stages for kb1-3 interleaved behind them
            for c in range(NC_CHUNKS):
                nc.sync.dma_start(wv_sb[:, c * 512:(c + 1) * 512],
                                  wv[c * 128:(c + 1) * 128, :])
            vt_sb = [vt_pool.tile([128, S], BF16, tag=f"vt{c}", name=f"vt{c}")
                     for c in range(NC_CHUNKS)]
            for tb in range(NQB):
                for c in range(NC_CHUNKS):
                    nc.sync.dma_start(
                        vt_sb[c][:, tb * 512:(tb + 1) * 512],
                        vT[c * 128:(c + 1) * 128, tb * 512:(tb + 1) * 512])
                if tb + 1 < NQB:
                    kstg[tb + 1] = stage_block(kT, tb + 1)
            for p in range(NP):
                nc.sync.dma_start(wo_sb[:, p * 1024:(p + 1) * 1024],
                                  wo[p * 128:(p + 1) * 128, :])

            # ---- attention + output projection ----
            # Software-pipelined over flat units u = (qb, p, kc): the
            # scores+exp issue runs LOOK units ahead of the mix issue so
            # ScalarE keeps exp-ing across pair boundaries while the PE
            # absorbs normalize/proj/Wo work in its slack.
            LOOK = 2
            units = [(qb, p, kc) for qb in range(NQB) for p in range(NP)
                     for kc in range(NKC)]
            pend = {}
            mix_tiles = {}
            stg_nxt = {}
            normT = {qb: [] for qb in range(NQB)}

            def issue_sc(u):
                qb, p, kc = u
                qsl = slice(qb * 512, (qb + 1) * 512)
                ksl = slice(kc * 128, (kc + 1) * 128)
                scP = sc_pool.tile([128, 1024], F32, tag="sc")
                # 64x128 PE row tiling: both heads co-stream
                nc.tensor.matmul(
                    scP[:, 0:512],
                    lhsT=khT[p][0:64, ksl], rhs=qhT[p][0:64, qsl],
                    start=True, stop=True, tile_position=(0, 0))
                nc.tensor.matmul(
                    scP[:, 512:1024],
                    lhsT=khT[p][64:128, ksl], rhs=qhT[p][64:128, qsl],
                    start=True, stop=True, tile_position=(64, 0))
                exP = exp_pool.tile([128, 1024], BF16, tag="exp")
                nc.scalar.activation(exP[:], scP[:], EXP)
                pend[u] = exP

            def issue_mix(u):
                qb, p, kc = u
                h0, h1 = 2 * p, 2 * p + 1
                if kc == 0:
                    mix_tiles[(qb, p)] = mx_pool.tile(
                        [128, 1024], F32, tag="mx", name="mixPR")
                mixPR = mix_tiles[(qb, p)]
                exP = pend.pop(u)
                va = vhs[kc]
                st = (kc == 0)
                sp = (kc == NKC - 1)
                nc.tensor.matmul(
                    mixPR[:, 0:512], lhsT=va[:, h0 * 128:(h0 + 1) * 128],
                    rhs=exP[:, 0:512], start=st, stop=sp)
                nc.tensor.matmul(
                    mixPR[:, 512:1024], lhsT=va[:, h1 * 128:(h1 + 1) * 128],
                    rhs=exP[:, 512:1024], start=st, stop=sp)

            def normalize(qb, p):
                mixPR = mix_tiles.pop((qb, p))
                nt = norm_pool.tile([128, 512], BF16, tag="norm")
                normT[qb].append(nt)
                if qb == NQB - 1 and p == NP - 1:
                    # final pair: shortest chain, straight from PSUM with a
                    # K=1 PE broadcast of the Z row (banks never reused)
                    zrow = tmp_pool.tile([128, 1024], BF16, tag="zb")
                    nc.vector.tensor_copy(zrow[64:65, :], mixPR[64:65, :])

                    def finish_last():
                        bc0 = pj_pool.tile([128, 512], F32, tag="pj")
                        bc1 = pj_pool.tile([128, 512], F32, tag="pj")
                        nc.tensor.matmul(
                            bc0[0:64, :], lhsT=e65_sb[64:65, 0:64],
                            rhs=zrow[64:65, 0:512], start=True, stop=True,
                            tile_position=(64, 0))
                        nc.tensor.matmul(
                            bc1[0:64, :], lhsT=e65_sb[64:65, 0:64],
                            rhs=zrow[64:65, 512:1024], start=True,
                            stop=True, tile_position=(64, 0))
                        recB = tmp_pool.tile([64, 1024], F32, tag="zb")
                        nc.vector.reciprocal_approx_fast(recB[:, 0:512],
                                                         bc0[0:64, :])
                        nc.vector.reciprocal_approx_fast(recB[:, 512:1024],
                                                         bc1[0:64, :])
                        nc.vector.tensor_mul(nt[0:64, :],
                                             mixPR[0:64, 0:512],
                                             recB[:, 0:512])
                        sh1 = tmp_pool.tile([64, 512], BF16, tag="sh1")
                        nc.vector.tensor_mul(sh1[:],
                                             mixPR[0:64, 512:1024],
                                             recB[:, 512:1024])
                        nc.sync.dma_start(nt[64:128, :], sh1[:])
                    return finish_last
                # evac mix PSUM to SBUF immediately (frees the banks fast)
                mloc = tmp_pool.tile([128, 1024], F32, tag="mloc")
                nc.vector.tensor_copy(mloc[:], mixPR[:])

                def finish():
                    # Z row -> partition 0 (DMA hop; partition_broadcast
                    # reads physical partition 0), bcast across partitions
                    # on idle GpSimd, then reciprocal + scale on DVE
                    zr0 = tmp_pool.tile([1, 1024], F32, tag="zr0")
                    nc.sync.dma_start(zr0[:], mloc[64:65, :])
                    zb = tmp_pool.tile([64, 1024], F32, tag="zb")
                    nc.gpsimd.partition_broadcast(zb[:], zr0[:])
                    rec = zb
                    nc.vector.reciprocal_approx_fast(rec[:], zb[:])
                    nc.vector.tensor_mul(nt[0:64, :], mloc[0:64, 0:512],
                                         rec[:, 0:512])
                    sh1 = tmp_pool.tile([64, 512], BF16, tag="sh1")
                    nc.vector.tensor_mul(sh1[:], mloc[0:64, 512:1024],
                                         rec[:, 512:1024])
                    nc.sync.dma_start(nt[64:128, :], sh1[:])
                return finish

            deferred = []

            def wo_piece(qb, tt, dh, half):
                def run():
                    key = ("wops", qb, tt, dh)
                    if half == 0:
                        mix_tiles[key] = pj_pool.tile(
                            [128, 512], F32, tag="pj", name="wps")
                    wps = mix_tiles[key]
                    for p in (0, 1) if half == 0 else (2, 3):
                        nc.tensor.matmul(
                            wps[:],
                            lhsT=normT[qb][p][:, tt * 128:(tt + 1) * 128],
                            rhs=wo_sb[:, p * 1024 + dh * 512:
                                      p * 1024 + (dh + 1) * 512],
                            start=(p == 0), stop=(p == NP - 1))
                    if half == 1:
                        del mix_tiles[key]
                        osb = out_pool.tile([128, 512], F32, tag="osb",
                                            name="osb")
                        nc.vector.tensor_copy(osb[:], wps[:])
                        nc.sync.dma_start(
                            out[qb * 512 + tt * 128:
                                qb * 512 + (tt + 1) * 128,
                                dh * 512:(dh + 1) * 512], osb[:])
                return run

            def proj_piece(qb, p, cs):
                def run():
                    stg = stg_nxt[qb]
                    key = ("pps", qb, p)
                    if cs == 0:
                        mix_tiles[key] = pj_pool.tile(
                            [128, 512], F32, tag="pj", name="pps")
                    ps = mix_tiles[key]
                    for c in (cs, cs + 1):
                        nc.tensor.matmul(
                            ps[:],
                            lhsT=wq_sb[:, c * 512 + p * 128:
                                       c * 512 + (p + 1) * 128],
                            rhs=stg[c][:],
                            start=(c == 0), stop=(c == NC_CHUNKS - 1))
                    if cs + 2 == NC_CHUNKS:
                        nc.vector.tensor_copy(
                            qhT[p][:, qb * 512:(qb + 1) * 512], ps[:])
                        del mix_tiles[key]
                return run

            kdone = {(p, 0) for p in range(NP)}
            for j in range(LOOK):
                issue_sc(units[j])
            for p in range(1, NP):
                proj_pair(qstg, wq_sb, p, qhT, 0)
                proj_pair(kstg[0], wk_sb, p, khT, 0)
            for i, u in enumerate(units):
                if u[0] == 0 and u[1] == 0:
                    vproj_granule(u[2])
                issue_mix(u)
                if i + LOOK < len(units):
                    nxt = units[i + LOOK]
                    if nxt[0] == 0 and nxt[2] % 4 == 0:
                        kk = (nxt[1], nxt[2] // 4)
                        if kk not in kdone:
                            kdone.add(kk)
                            proj_pair(kstg[kk[1]], wk_sb, kk[0], khT, kk[1])
                    issue_sc(nxt)
                if deferred:
                    deferred.pop(0)()
                qb, p, kc = u
                if p == 0 and kc == 14 and qb + 1 < NQB:
                    stg_nxt[qb + 1] = stage_block(qT, qb + 1)
                if kc == NKC - 1:
                    deferred.append(normalize(qb, p))
                    if qb + 1 < NQB:
                        deferred.extend(
                            proj_piece(qb + 1, p, cs)
                            for cs in range(0, NC_CHUNKS, 2))
                    if p == NP - 1:
                        deferred.extend(wo_piece(qb, tt, dh, half)
                                        for tt in range(4) for dh in range(2)
                                        for half in range(2))
            while deferred:
                deferred.pop(0)()

    nc.compile()
    return nc


def _get_nc():
    if "nc" not in _COMPILED:
        _COMPILED["nc"] = _build_nc()
    return _COMPILED["nc"]


def _shard_inputs(q, k, v, mask, Wq, Wk, Wv, Wo):
    """Build the per-core input maps (host-side layout prep)."""
    import ml_dtypes

    bf16 = ml_dtypes.bfloat16
    in_maps = []
    maskf = np.asarray(mask).astype(np.float32)
    q = np.asarray(q, np.float32)
    k = np.asarray(k, np.float32)
    v = np.asarray(v, np.float32)
    Wq = np.asarray(Wq, np.float32)
    Wk = np.asarray(Wk, np.float32)
    Wv = np.asarray(Wv, np.float32)
    Wo = np.asarray(Wo, np.float32)
    scale = np.float32(1.0 / np.sqrt(DK))
    for c in range(NCORES):
        b, hg = c // 2, c % 2
        hs = hg * HC
        m = {
            "qT": np.ascontiguousarray(q[b].T).astype(bf16),
            "kT": np.ascontiguousarray(k[b].T).astype(bf16),
            "vT": np.ascontiguousarray(v[b].T).astype(bf16),
            # head-major col blocks; fold 1/sqrt(dk) into Wq
            "wq": np.ascontiguousarray(
                Wq[hs:hs + HC].transpose(1, 0, 2).reshape(D, HC * DK) * scale
            ).astype(bf16),
            "wk": np.ascontiguousarray(
                Wk[hs:hs + HC].transpose(1, 0, 2).reshape(D, HC * DK)
            ).astype(bf16),
            "wv": np.ascontiguousarray(
                Wv[hs:hs + HC].transpose(1, 0, 2).reshape(D, HC * DV)
            ).astype(bf16),
            "wo": np.ascontiguousarray(Wo[hs * DV:(hs + HC) * DV]).astype(bf16),
            "maskr": np.ascontiguousarray(
                maskf[b].reshape(NKC, 128).T).astype(np.float32),
        }
        in_maps.append(m)
    return in_maps


def kernel(q, k, v, mask, Wq, Wk, Wv, Wo, _trace=False):
    from concourse.bass_utils import run_bass_kernel_spmd

    nc = _get_nc()
    in_maps = _shard_inputs(q, k, v, mask, Wq, Wk, Wv, Wo)
    res = run_bass_kernel_spmd(nc, in_maps, list(range(NCORES)),
                               trace=_trace)
    out = np.zeros((B, S, D), np.float32)
    for c in range(NCORES):
        out[c // 2] += res.results[c]["out"]
    if _trace:
        _COMPILED["last_result"] = res
    return out
